# revision 9
# baseline (speedup 1.0000x reference)
"""Multi-head causal attention (B=4,S=2048,D=768,H=12,HD=64) on 8 Trainium2 cores.

Sharding: 4-way head tensor-parallel (3 heads/core) x 2-way batch data-parallel
(2 batches/core).  Core c: batch group bg=c//4 (batches 2bg,2bg+1), head group
hg=c%4 (heads 3hg..3hg+2).

Per-core device program (SPMD; per-core differences come only from data):
  1. q/k projections emitted transposed (qT,kT: [64 head-dim partitions, rows]);
     v projection row-major with 64 appended ones columns per head (softmax
     denominator rides along the AV matmul replicated on psum rows 64:128).
  2. Causal attention computed transposed: S_T[k,q] = kT.T @ qT, so P=exp(S_T)
     feeds AV directly with no P transpose.  Softmax skips the running max
     (scores are O(1) at this problem's scale; exp is mathematically identical
     to the reference since softmax is shift-invariant).  The two batches of a
     head run concurrently on the PE via 64-row tile packing (row-tiled
     matmuls).  AV accumulates ctxU_T[128, q512] = sum_k vE.T @ P_T (rows
     64:128 = denominator l replicated).  Normalize: 1/l via fast-approx DVE
     reciprocal, one fused DVE multiply -- no partition broadcast, no
     single-partition ops.
  3. Per 512-row q-block (x2 batches = 1024-row chunk): 8-core AllToAll (bf16,
     128-row shards) redistributes ctx so each core holds all 768 context
     features for its own 2x128 output rows; local projection with full Wp;
     bias folded into the psum->sbuf copy as a DVE tensor-add.

The whole attention sweep is software-pipelined one k-pair deep: the score
matmuls of unit t+1 are emitted before the exp/AV of unit t, so the PE chews
on S_T[t+1] while the scalar engine runs EXP[t].  qk/v projection chunks and
out-projection blocks are interleaved between units as PE filler.

Tail: q-block 3 runs its pairs in order (2,0,1) and ships each pair's context
in its own small AllToAll right after the pair finishes (B=pair2, A0=pair0,
A1=pair1).  Its out-projection accumulates in SBUF in three slices -- the
pair2+pair0 slice runs as filler during pair-1's attention, so after the
final (A1) collective only two 2-matmul completions + adds + stores remain.
ag fetches are emitted immediately after the collective that produces them
(DMAs emitted later on the same queues would be gated on the collective's
completion semaphore -- keep MM fillers separate from fetches).

Host side only slices/casts/transposes inputs and concatenates output shards.
"""

import sys

if "/opt/trn_rl_repo" not in sys.path:
    sys.path.insert(0, "/opt/trn_rl_repo")

import numpy as np
import ml_dtypes

BF16 = ml_dtypes.bfloat16

B, S, D = 4, 2048, 768
H, HD = 12, 64
N_CORES = 8
BL = 2          # batches per core
HL = 3          # heads per core
R = BL * S      # 4096 rows per core
KSUB = D // 128  # 6

_CACHE = {}


def _build_nc():
    import concourse.bass as bass  # noqa: F401
    import concourse.tile as tile
    from concourse import bacc, mybir

    f32 = mybir.dt.float32
    bf16 = mybir.dt.bfloat16
    EXP = mybir.ActivationFunctionType.Exp

    nc = bacc.Bacc("TRN2", target_bir_lowering=False, debug=False,
                   num_devices=N_CORES)

    xT_d = nc.dram_tensor("xT", [D, R], bf16, kind="ExternalInput").ap()
    wqk_d = nc.dram_tensor("wqk", [D, 2 * HL * HD], bf16, kind="ExternalInput").ap()
    wv_d = nc.dram_tensor("wv", [D, HL * HD], bf16, kind="ExternalInput").ap()
    wp_d = nc.dram_tensor("wp", [D, D], bf16, kind="ExternalInput").ap()
    wp2_d = nc.dram_tensor("wp2", [D, D], bf16, kind="ExternalInput").ap()
    bp_d = nc.dram_tensor("bp", [1, D], bf16, kind="ExternalInput").ap()
    mk_d = nc.dram_tensor("mk", [2, 128, 1024], bf16, kind="ExternalInput").ap()
    out_d = nc.dram_tensor("out", [4, 2, 128, D], f32, kind="ExternalOutput").ap()

    RG = [[0, 1, 2, 3, 4, 5, 6, 7]]

    with tile.TileContext(nc) as tc:
        with tc.tile_pool(name="persist", bufs=1) as per, \
             tc.tile_pool(name="dram", bufs=1, space="DRAM") as dram, \
             tc.tile_pool(name="mix_ps", bufs=2, space="PSUM") as mix_ps, \
             tc.tile_pool(name="st_ps", bufs=2, space="PSUM") as st_ps, \
             tc.tile_pool(name="av_ps", bufs=2, space="PSUM") as av_ps, \
             tc.tile_pool(name="pt", bufs=8) as ptp, \
             tc.tile_pool(name="sm", bufs=4) as sm, \
             tc.tile_pool(name="agp", bufs=4) as agp, \
             tc.tile_pool(name="outp", bufs=4) as outp:
            # ---- persistent SBUF tensors -------------------------------
            wqk = per.tile([128, KSUB, 2 * HL * HD], bf16, tag="wqk")
            wv = per.tile([128, KSUB, HL * HD], bf16, tag="wv")
            xT = per.tile([128, KSUB, R], bf16, tag="xT")
            xTr = xT_d.rearrange("(o p) r -> p o r", p=128)
            for j in range(KSUB):  # consumption order for fast PE start
                nc.sync.dma_start(
                    wqk[:, j], wqk_d.rearrange("(o p) c -> p o c", p=128)[:, j])
                nc.sync.dma_start(xT[:, j, 0:512], xTr[:, j, 0:512])
                nc.sync.dma_start(xT[:, j, S:S + 512], xTr[:, j, S:S + 512])
                nc.sync.dma_start(
                    wv[:, j], wv_d.rearrange("(o p) c -> p o c", p=128)[:, j])
            # remaining x chunks, alternating batches (b0rc, b1rc)
            for rc in range(1, 4):
                for b in range(2):
                    r0 = b * S + rc * 512
                    for j in range(KSUB):
                        nc.sync.dma_start(xT[:, j, r0:r0 + 512],
                                          xTr[:, j, r0:r0 + 512])
            masks = per.tile([128, 2, 1024], bf16, tag="mk")
            nc.sync.dma_start(masks[:], mk_d.rearrange("o p c -> p o c"))
            wp = per.tile([128, KSUB, D], bf16, tag="wp")
            nc.sync.dma_start(wp[:], wp_d.rearrange("(o p) c -> p o c", p=128))
            wp2 = per.tile([128, KSUB, D], bf16, tag="wp2")
            nc.sync.dma_start(wp2[:], wp2_d.rearrange("(o p) c -> p o c", p=128))
            bp_sb = per.tile([1, D], bf16, tag="bp")
            nc.sync.dma_start(bp_sb[:], bp_d[:])
            biasB = per.tile([128, D], bf16, tag="biasB")
            nc.gpsimd.partition_broadcast(biasB[:], bp_sb[:])

            # pair p = head p; partitions 0:64 = batch 0, 64:128 = batch 1
            qT = per.tile([128, HL, S], bf16, tag="qT")
            kT = per.tile([128, HL, S], bf16, tag="kT")
            # vE per (row-tile, pair): [v 64 cols | ones 64 cols] so the AV
            # matmul (M=128) lands the denominator replicated on psum
            # partitions 64:128
            vE = per.tile([128, 2 * 16, HL, 128], bf16, tag="vE")
            nc.vector.memset(vE[:], 1.0)

            warm_in = dram.tile([8 * 192, 8], bf16, tag="warm_in")
            warm_out = dram.tile([8 * 192, 8], bf16, tag="warm_out")
            a2a_in = [dram.tile([8 * 128, 128], bf16, name=f"a2ai{qb}",
                                tag=f"a2ai{qb}") for qb in range(3)]
            a2a_out = [dram.tile([8 * 128, 128], bf16, name=f"a2ao{qb}",
                                 tag=f"a2ao{qb}") for qb in range(3)]
            a2b_in = [dram.tile([8 * 64, 128], bf16, name=f"a2bi{qb}",
                                tag=f"a2bi{qb}") for qb in range(4)]
            a2b_out = [dram.tile([8 * 64, 128], bf16, name=f"a2bo{qb}",
                                 tag=f"a2bo{qb}") for qb in range(4)]
            a2c_in = [dram.tile([8 * 64, 128], bf16, name=f"a2ci{p}",
                                tag=f"a2ci{p}") for p in range(2)]
            a2c_out = [dram.tile([8 * 64, 128], bf16, name=f"a2co{p}",
                                 tag=f"a2co{p}") for p in range(2)]

            # ---- emission helpers --------------------------------------
            def emit_qk_ct(b, rc, ct):
                r0 = b * S + rc * 512
                if True:
                    ps = mix_ps.tile([128, 512], f32, tag="mix", name="ps")
                    for j in range(KSUB):
                        nc.tensor.matmul(
                            ps[:],
                            lhsT=wqk[:, j, ct * 128:(ct + 1) * 128],
                            rhs=xT[:, j, r0:r0 + 512],
                            start=(j == 0), stop=(j == KSUB - 1))
                    for half in range(2):
                        gid = 2 * ct + half
                        dest = qT if gid < 3 else kT
                        pair = gid % 3
                        nc.vector.tensor_copy(
                            dest[b * 64:(b + 1) * 64, pair,
                                 rc * 512:(rc + 1) * 512],
                            ps[half * 64:(half + 1) * 64, :])

            def emit_v_tile(b, rt):
                r0 = b * S + rt * 128
                psv = mix_ps.tile([128, HL * HD], f32, tag="mix", name="psv")
                for j in range(KSUB):
                    nc.tensor.matmul(
                        psv[:], lhsT=xT[:, j, r0:r0 + 128], rhs=wv[:, j, :],
                        start=(j == 0), stop=(j == KSUB - 1))
                nc.vector.tensor_copy(
                    vE[:, b * 16 + rt, :, 0:HD],
                    psv[:, :].rearrange("p (h c) -> p h c", c=HD))

            # ---- attention units (qb, pair, kp), software-pipelined ----
            PAIR_ORDER = {0: [0, 1, 2], 1: [0, 1, 2], 2: [0, 1, 2],
                          3: [2, 0, 1]}
            units = [(qb, pair, kp)
                     for qb in range(4)
                     for pair in PAIR_ORDER[qb]
                     for kp in range(2 * (qb + 1))]
            NU = len(units)
            uidx = {u: i for i, u in enumerate(units)}
            st_tiles = {}   # t -> [stps_u0, stps_u1]
            av_tiles = {}   # (qb, pair) -> [avs_u0, avs_u1]

            def emit_st(t):
                qb, pair, kp = units[t]
                n_kp = 2 * (qb + 1)
                q0 = qb * 512
                o = kp - (n_kp - 2)  # diag pair offset; >=0 on diagonal
                qv0 = 256 if o == 1 else 0
                stps = [st_ps.tile([128, 2, 512], f32, tag="st",
                                   name=f"st{u}") for u in range(2)]
                st_tiles[t] = stps
                for i in range(2):
                    for u in range(2):
                        kt = 2 * kp + i
                        nc.tensor.matmul(
                            stps[u][:, i, qv0:512],
                            lhsT=kT[u * 64:(u + 1) * 64, pair,
                                    kt * 128:(kt + 1) * 128],
                            rhs=qT[u * 64:(u + 1) * 64, pair,
                                   q0 + qv0:q0 + 512],
                            start=True, stop=True)

            def emit_sm_av(t):
                qb, pair, kp = units[t]
                n_kp = 2 * (qb + 1)
                o = kp - (n_kp - 2)
                qv0 = 256 if o == 1 else 0
                if kp == 0:
                    av_tiles[(qb, pair)] = [
                        av_ps.tile([128, 512], f32, tag="av", name=f"av{u}")
                        for u in range(2)]
                avs = av_tiles[(qb, pair)]
                stps = st_tiles.pop(t)
                for u in range(2):
                    pt = ptp.tile([128, 2, 512], bf16, tag="pt")
                    nc.scalar.activation(pt[:, :, qv0:512],
                                         stps[u][:, :, qv0:512], EXP,
                                         scale=float(HD) ** -0.5)
                    if o >= 0:
                        mk2 = masks[:, o, :].rearrange("p (i c) -> p i c",
                                                       i=2)
                        nc.vector.tensor_mul(pt[:, :, qv0:512],
                                             pt[:, :, qv0:512],
                                             mk2[:, :, qv0:512])
                    for i in range(2):
                        kt = 2 * kp + i
                        nc.tensor.matmul(
                            avs[u][:, qv0:512],
                            lhsT=vE[:, u * 16 + kt, pair, :],
                            rhs=pt[:, i, qv0:512],
                            start=(kp == 0 and i == 0),
                            stop=(kp == n_kp - 1 and i == 1))

            def emit_norm(qb, pair):
                avs = av_tiles.pop((qb, pair))
                for u in range(2):
                    lsb = sm.tile([64, 512], f32, tag="lsb", name=f"lsb{u}")
                    nc.vector.tensor_copy(lsb[:], avs[u][64:128, :])
                    rec = sm.tile([64, 512], f32, tag="rec", name=f"rec{u}")
                    nc.vector.reciprocal_approx_fast(rec[:], lsb[:])
                    ctxn = sm.tile([64, 512], bf16, tag="ctxn",
                                   name=f"ctxn{u}")
                    nc.vector.tensor_mul(ctxn[:], avs[u][0:64, :], rec[:])
                    if qb < 3:
                        if pair < 2:
                            a2v = a2a_in[qb].rearrange("(j f) c -> f j c",
                                                       f=128)
                            a2v = a2v[64 * pair:64 * (pair + 1), :, :]
                        else:
                            a2v = a2b_in[qb].rearrange("(j f) c -> f j c",
                                                       f=64)
                    else:
                        dst = a2b_in[3] if pair == 2 else a2c_in[pair]
                        a2v = dst.rearrange("(j f) c -> f j c", f=64)
                    nc.sync.dma_start(
                        a2v[:, 4 * u:4 * u + 4, :],
                        ctxn.rearrange("p (q c) -> p q c", q=4))

            def emit_coll(ins, outs):
                nc.gpsimd.collective_compute(
                    "AllToAll", mybir.AluOpType.bypass,
                    ins=[ins[:]], outs=[outs[:]], replica_groups=RG)

            def fetch_ag(ag, qb):
                nc.sync.dma_start(
                    ag[:, 0:8, :],
                    a2a_out[qb].rearrange("(o p) r -> p o r", p=128))
                nc.sync.dma_start(
                    ag[:, 8:12, :],
                    a2b_out[qb].rearrange("(o p) r -> p o r", p=128))

            def emit_outproj_blk(qb, blk, ag):
                osb = outp.tile([128, D], f32, tag="osb")
                oblk = [blk * 4 + t for t in range(4)] + \
                    [8 + 2 * blk, 9 + 2 * blk]
                for nh in range(2):
                    po = mix_ps.tile([128, 384], f32, tag="mix", name="po")
                    n0 = nh * 384
                    for j in range(KSUB):
                        nc.tensor.matmul(po[:],
                                         lhsT=ag[:, oblk[j], :],
                                         rhs=wp[:, j, n0:n0 + 384],
                                         start=(j == 0),
                                         stop=(j == KSUB - 1))
                    nc.vector.tensor_add(osb[:, n0:n0 + 384], po[:],
                                         biasB[:, n0:n0 + 384])
                nc.sync.dma_start(out_d[qb, blk], osb[:])

            # qb3 out-projection: ag3 chunk layout 0:4 = pair0 (a2c0),
            # 4:8 = pair1 (a2c1), 8:12 = pair2 (a2b3); wp2 row-chunks
            # j0..j5 = [hg01 p0][hg23 p0][hg01 p1][hg23 p1][hg01 p2][hg23 p2]
            def emit_op3_part1(blk, nh, ag3, osb3):
                po = mix_ps.tile([128, 384], f32, tag="mix", name="po")
                n0 = nh * 384
                chunks = [(8 + 2 * blk, 4), (9 + 2 * blk, 5),
                          (2 * blk, 0), (2 * blk + 1, 1)]
                for n, (c, j) in enumerate(chunks):
                    nc.tensor.matmul(po[:], lhsT=ag3[:, c, :],
                                     rhs=wp2[:, j, n0:n0 + 384],
                                     start=(n == 0), stop=(n == 3))
                nc.vector.tensor_add(osb3[blk][:, n0:n0 + 384], po[:],
                                     biasB[:, n0:n0 + 384])

            # ---- software-pipelined emission ---------------------------
            # warmup collective: absorb ncfw first-call overhead during proj
            nc.sync.dma_start(warm_in[0:128, :], masks[:, 0, 0:8])
            emit_coll(warm_in, warm_out)
            # prologue: everything attention qb0 needs
            for ct in range(3):
                emit_qk_ct(0, 0, ct)
            for ct in range(3):
                emit_qk_ct(1, 0, ct)
            for rt in range(4):
                emit_v_tile(0, rt)
                emit_v_tile(1, rt)

            # filler queue: (deadline=emission step, emit_fn), kept in
            # deadline order; before each unit's S_T all units due by then
            # are drained (hard ordering requirement: a unit must be emitted
            # before the attention that consumes its output), plus one unit
            # opportunistically per step to spread PE filler.
            import heapq
            fqh = []
            fqseq = [0]

            def fq_push(dl, fn):
                heapq.heappush(fqh, (dl, fqseq[0], fn))
                fqseq[0] += 1

            for rc in range(1, 4):
                p0 = PAIR_ORDER[rc][0]
                for b in range(2):
                    for ct in range(3):
                        # qT/kT rows rc needed from the first unit of qb=rc;
                        # its S_T is emitted one step early
                        fq_push(uidx[(rc, p0, 0)] - 1,
                                lambda b=b, rc=rc, ct=ct:
                                emit_qk_ct(b, rc, ct))
                for rt in range(4 * rc, 4 * rc + 4):
                    for b in range(2):
                        # vE row-tile rt consumed at AV of kp=rt//2 of the
                        # first pair of qb=rc
                        fq_push(uidx[(rc, p0, max(0, rt // 2 - 1))],
                                lambda b=b, rt=rt: emit_v_tile(b, rt))

            def drain(n, due=None):
                k = 0
                while fqh and (k < n or (due is not None and fqh[0][0] <= due)):
                    heapq.heappop(fqh)[2]()
                    k += 1

            ags = {}
            osb3 = []

            emit_st(0)
            for t in range(NU):
                if t + 1 < NU:
                    drain(1, due=t + 1)
                    emit_st(t + 1)
                emit_sm_av(t)
                qb, pair, kp = units[t]
                if kp != 2 * (qb + 1) - 1:
                    continue
                # ---- last k-pair of (qb, pair) ----
                emit_norm(qb, pair)
                if qb < 3 and pair == 1:
                    emit_coll(a2a_in[qb], a2a_out[qb])
                if qb < 3 and pair == 2:
                    # end of q-block qb: ship pair2, then immediately fetch
                    # ag(qb) (gated on B(qb) completing -- which it needs
                    # anyway); out-proj blocks run as filler ~1 q-block later
                    emit_coll(a2b_in[qb], a2b_out[qb])
                    ag = agp.tile([128, 2 * KSUB, 128], bf16, tag="ag",
                                  name=f"ag{qb}")
                    ags[qb] = ag
                    fetch_ag(ag, qb)
                    sched = {0: [(2, 0, 2), (2, 0, 5)],
                             1: [(2, 1, 2), (2, 1, 5)],
                             2: [(3, 2, 5), (3, 2, 7)]}[qb]
                    for blk, su in enumerate(sched):
                        fq_push(uidx[su], lambda qb=qb, blk=blk, a=ag:
                                emit_outproj_blk(qb, blk, a))
                if qb == 3 and pair == 2:   # first pair of q-block 3
                    emit_coll(a2b_in[3], a2b_out[3])
                    ag3 = agp.tile([128, 2 * KSUB, 128], bf16, tag="ag",
                                   name="ag3")
                    nc.sync.dma_start(
                        ag3[:, 8:12, :],
                        a2b_out[3].rearrange("(o p) r -> p o r", p=128))
                if qb == 3 and pair == 0:
                    emit_coll(a2c_in[0], a2c_out[0])
                    nc.sync.dma_start(
                        ag3[:, 0:4, :],
                        a2c_out[0].rearrange("(o p) r -> p o r", p=128))
                    osb3 = [outp.tile([128, D], f32, tag="osb",
                                      name=f"osb3_{blk}") for blk in range(2)]
                    for n, (blk, nh) in enumerate(
                            [(0, 0), (0, 1), (1, 0), (1, 1)]):
                        fq_push(uidx[(3, 1, 3 + n)],
                                lambda blk=blk, nh=nh:
                                emit_op3_part1(blk, nh, ag3, osb3))
                if qb == 3 and pair == 1:   # last unit of the kernel
                    drain(99, due=NU)
                    emit_coll(a2c_in[1], a2c_out[1])
            # ---- epilogue: only the pair-1 slice remains ----------------
            nc.sync.dma_start(
                ag3[:, 4:8, :],
                a2c_out[1].rearrange("(o p) r -> p o r", p=128))
            for blk in range(2):
                for nh in range(2):
                    po = mix_ps.tile([128, 384], f32, tag="mix", name="po")
                    n0 = nh * 384
                    nc.tensor.matmul(po[:], lhsT=ag3[:, 4 + 2 * blk, :],
                                     rhs=wp2[:, 2, n0:n0 + 384],
                                     start=True, stop=False)
                    nc.tensor.matmul(po[:], lhsT=ag3[:, 5 + 2 * blk, :],
                                     rhs=wp2[:, 3, n0:n0 + 384],
                                     start=False, stop=True)
                    nc.vector.tensor_add(osb3[blk][:, n0:n0 + 384], po[:],
                                         osb3[blk][:, n0:n0 + 384])
                nc.sync.dma_start(out_d[3, blk], osb3[blk][:])

    nc.compile()
    return nc


def _get_nc():
    if "nc" not in _CACHE:
        _CACHE["nc"] = _build_nc()
    return _CACHE["nc"]


def _masks_np():
    k = np.arange(128)[:, None]
    q = np.arange(512)[None, :]
    tiles = [(q >= k + 128 * t) for t in range(4)]
    m = np.stack([np.concatenate([tiles[2 * o], tiles[2 * o + 1]], axis=1)
                  for o in range(2)])
    return m.astype(BF16)


def _prep_in_maps(x, Wq, Wk, Wv, Wp, bp):
    x = np.asarray(x, dtype=np.float32)
    mk = _masks_np()
    # Wp rows permuted to match the split-AllToAll layout: first each head
    # group's pair-0/1 features (4 x 128 rows), then all pair-2 features
    # (4 x 64 rows) — keeps every received region 128-row aligned.
    wpa = np.asarray(Wp)
    wp_full = np.concatenate(
        [wpa[192 * h:192 * h + 128] for h in range(4)]
        + [wpa[192 * h + 128:192 * h + 192] for h in range(4)],
        axis=0).astype(BF16)
    # q-block-3 layout: per-pair AllToAlls -> rows grouped per pair:
    # [hg0 p][hg1 p][hg2 p][hg3 p] for p = 0, 1, 2
    wp2_full = np.concatenate(
        [wpa[192 * h + 64 * p:192 * h + 64 * p + 64]
         for p in range(3) for h in range(4)],
        axis=0).astype(BF16)
    bp_row = np.asarray(bp, dtype=np.float32).reshape(1, D).astype(BF16)
    xT_bg = []
    for bg in range(2):
        xl = x[2 * bg:2 * bg + 2].reshape(R, D)
        xT_bg.append(np.ascontiguousarray(xl.T).astype(BF16))
    wqk_hg, wv_hg = [], []
    for hg in range(4):
        hs = slice(192 * hg, 192 * (hg + 1))
        wqk_hg.append(np.concatenate(
            [np.asarray(Wq)[:, hs], np.asarray(Wk)[:, hs]], axis=1).astype(BF16))
        wv_hg.append(np.asarray(Wv)[:, hs].astype(BF16))
    in_maps = []
    for c in range(N_CORES):
        bg, hg = c // 4, c % 4
        in_maps.append({
            "xT": xT_bg[bg],
            "wqk": wqk_hg[hg],
            "wv": wv_hg[hg],
            "wp": wp_full,
            "wp2": wp2_full,
            "bp": bp_row,
            "mk": mk,
        })
    return in_maps


def kernel(x, Wq, Wk, Wv, Wp, bp):
    from concourse import bass_utils

    nc = _get_nc()
    in_maps = _prep_in_maps(x, Wq, Wk, Wv, Wp, bp)
    res = bass_utils.run_bass_kernel_spmd(nc, in_maps,
                                          core_ids=list(range(N_CORES)))
    out = np.empty((B, S, D), np.float32)
    for c in range(N_CORES):
        sh = res.results[c]["out"]  # [4 chunks, 2 blocks, 128, D]
        for qb in range(4):
            for blk in range(2):
                batch = 2 * blk + c // 4
                s0 = 512 * qb + 128 * (c % 4)
                out[batch, s0:s0 + 128] = sh[qb, blk]
    return out


# revision 10
# speedup vs baseline: 1.0957x; 1.0957x over previous
"""Multi-head causal attention (B=4,S=2048,D=768,H=12,HD=64) on 8 Trainium2 cores.

Sharding: 4-way head tensor-parallel (3 heads/core) x 2-way batch data-parallel
(2 batches/core).  Core c: batch group bg=c//4 (batches 2bg,2bg+1), head group
hg=c%4 (heads 3hg..3hg+2).

Per-core device program (SPMD; per-core differences come only from data):
  1. q/k projections emitted transposed (qT,kT: [64 head-dim partitions, rows]);
     v projection row-major with 64 appended ones columns per head (softmax
     denominator rides along the AV matmul replicated on psum rows 64:128).
  2. Causal attention computed transposed: S_T[k,q] = kT.T @ qT, so P=exp(S_T)
     feeds AV directly with no P transpose.  Softmax skips the running max
     (scores are O(1) at this problem's scale; exp is mathematically identical
     to the reference since softmax is shift-invariant).  The two batches of a
     head run concurrently on the PE via 64-row tile packing (row-tiled
     matmuls).  AV accumulates ctxU_T[128, q512] = sum_k vE.T @ P_T (rows
     64:128 = denominator l replicated).  Normalize: 1/l via fast-approx DVE
     reciprocal, one fused DVE multiply -- no partition broadcast, no
     single-partition ops.
  3. Per 512-row q-block (x2 batches = 1024-row chunk): ONE 8-core AllToAll
     (bf16, 192-row shards = all 3 head-pairs) at block end redistributes ctx
     so each core holds all 768 context features for its own 2x128 output
     rows; 4 senders x 192 rows = 768 = 6x128, so the received buffer carves
     into K=128 out-projection chunks that line up with the NATURAL Wp row
     order (no permutation); bias folds into the psum->sbuf copy as a DVE
     tensor-add.

The whole attention sweep is software-pipelined one k-pair deep: the score
matmuls of unit t+1 are emitted before the exp/AV of unit t, so the PE chews
on S_T[t+1] while the scalar engine runs EXP[t].  qk/v projection chunks and
out-projection blocks are interleaved between units as PE filler, scheduled
so no DMA or matmul is ever queue-gated on an unfinished collective
(collectives serialize in emission order and gate everything emitted after
them on the same queues).  Only q-block 3's own out-projection trails the
final AllToAll, split per 384-wide half so stores overlap the matmuls.

Host side only slices/casts/transposes inputs and concatenates output shards.
"""

import sys

if "/opt/trn_rl_repo" not in sys.path:
    sys.path.insert(0, "/opt/trn_rl_repo")

import numpy as np
import ml_dtypes

BF16 = ml_dtypes.bfloat16

B, S, D = 4, 2048, 768
H, HD = 12, 64
N_CORES = 8
BL = 2          # batches per core
HL = 3          # heads per core
R = BL * S      # 4096 rows per core
KSUB = D // 128  # 6

_CACHE = {}


def _build_nc():
    import concourse.bass as bass  # noqa: F401
    import concourse.tile as tile
    from concourse import bacc, mybir

    f32 = mybir.dt.float32
    bf16 = mybir.dt.bfloat16
    EXP = mybir.ActivationFunctionType.Exp

    nc = bacc.Bacc("TRN2", target_bir_lowering=False, debug=False,
                   num_devices=N_CORES)

    xT_d = nc.dram_tensor("xT", [D, R], bf16, kind="ExternalInput").ap()
    wqk_d = nc.dram_tensor("wqk", [D, 2 * HL * HD], bf16, kind="ExternalInput").ap()
    wv_d = nc.dram_tensor("wv", [D, HL * HD], bf16, kind="ExternalInput").ap()
    wp_d = nc.dram_tensor("wp", [D, D], bf16, kind="ExternalInput").ap()
    bp_d = nc.dram_tensor("bp", [1, D], bf16, kind="ExternalInput").ap()
    mk_d = nc.dram_tensor("mk", [2, 128, 1024], bf16, kind="ExternalInput").ap()
    out_d = nc.dram_tensor("out", [4, 2, 128, D], f32, kind="ExternalOutput").ap()

    RG = [[0, 1, 2, 3, 4, 5, 6, 7]]

    with tile.TileContext(nc) as tc:
        with tc.tile_pool(name="persist", bufs=1) as per, \
             tc.tile_pool(name="dram", bufs=1, space="DRAM") as dram, \
             tc.tile_pool(name="mix_ps", bufs=2, space="PSUM") as mix_ps, \
             tc.tile_pool(name="st_ps", bufs=2, space="PSUM") as st_ps, \
             tc.tile_pool(name="av_ps", bufs=2, space="PSUM") as av_ps, \
             tc.tile_pool(name="pt", bufs=8) as ptp, \
             tc.tile_pool(name="sm", bufs=4) as sm, \
             tc.tile_pool(name="agp", bufs=4) as agp, \
             tc.tile_pool(name="outp", bufs=4) as outp:
            # ---- persistent SBUF tensors -------------------------------
            wqk = per.tile([128, KSUB, 2 * HL * HD], bf16, tag="wqk")
            wv = per.tile([128, KSUB, HL * HD], bf16, tag="wv")
            xT = per.tile([128, KSUB, R], bf16, tag="xT")
            xTr = xT_d.rearrange("(o p) r -> p o r", p=128)
            for j in range(KSUB):  # consumption order for fast PE start
                nc.sync.dma_start(
                    wqk[:, j], wqk_d.rearrange("(o p) c -> p o c", p=128)[:, j])
                nc.sync.dma_start(xT[:, j, 0:512], xTr[:, j, 0:512])
                nc.sync.dma_start(xT[:, j, S:S + 512], xTr[:, j, S:S + 512])
                nc.sync.dma_start(
                    wv[:, j], wv_d.rearrange("(o p) c -> p o c", p=128)[:, j])
            # remaining x chunks, alternating batches (b0rc, b1rc)
            for rc in range(1, 4):
                for b in range(2):
                    r0 = b * S + rc * 512
                    for j in range(KSUB):
                        nc.sync.dma_start(xT[:, j, r0:r0 + 512],
                                          xTr[:, j, r0:r0 + 512])
            masks = per.tile([128, 2, 1024], bf16, tag="mk")
            nc.sync.dma_start(masks[:], mk_d.rearrange("o p c -> p o c"))
            wp = per.tile([128, KSUB, D], bf16, tag="wp")
            nc.sync.dma_start(wp[:], wp_d.rearrange("(o p) c -> p o c", p=128))
            bp_sb = per.tile([1, D], bf16, tag="bp")
            nc.sync.dma_start(bp_sb[:], bp_d[:])
            biasB = per.tile([128, D], bf16, tag="biasB")
            nc.gpsimd.partition_broadcast(biasB[:], bp_sb[:])

            # pair p = head p; partitions 0:64 = batch 0, 64:128 = batch 1
            qT = per.tile([128, HL, S], bf16, tag="qT")
            kT = per.tile([128, HL, S], bf16, tag="kT")
            # vE per (row-tile, pair): [v 64 cols | ones 64 cols] so the AV
            # matmul (M=128) lands the denominator replicated on psum
            # partitions 64:128
            vE = per.tile([128, 2 * 16, HL, 128], bf16, tag="vE")
            nc.vector.memset(vE[:], 1.0)

            warm_in = dram.tile([8 * 192, 8], bf16, tag="warm_in")
            warm_out = dram.tile([8 * 192, 8], bf16, tag="warm_out")
            ab_in = [dram.tile([8 * 192, 128], bf16, name=f"abi{qb}",
                               tag=f"abi{qb}") for qb in range(4)]
            ab_out = [dram.tile([8 * 192, 128], bf16, name=f"abo{qb}",
                                tag=f"abo{qb}") for qb in range(4)]

            # ---- emission helpers --------------------------------------
            def emit_qk_ct(b, rc, ct):
                r0 = b * S + rc * 512
                if True:
                    ps = mix_ps.tile([128, 512], f32, tag="mix", name="ps")
                    for j in range(KSUB):
                        nc.tensor.matmul(
                            ps[:],
                            lhsT=wqk[:, j, ct * 128:(ct + 1) * 128],
                            rhs=xT[:, j, r0:r0 + 512],
                            start=(j == 0), stop=(j == KSUB - 1))
                    for half in range(2):
                        gid = 2 * ct + half
                        dest = qT if gid < 3 else kT
                        pair = gid % 3
                        nc.vector.tensor_copy(
                            dest[b * 64:(b + 1) * 64, pair,
                                 rc * 512:(rc + 1) * 512],
                            ps[half * 64:(half + 1) * 64, :])

            def emit_v_tile(b, rt):
                r0 = b * S + rt * 128
                psv = mix_ps.tile([128, HL * HD], f32, tag="mix", name="psv")
                for j in range(KSUB):
                    nc.tensor.matmul(
                        psv[:], lhsT=xT[:, j, r0:r0 + 128], rhs=wv[:, j, :],
                        start=(j == 0), stop=(j == KSUB - 1))
                nc.vector.tensor_copy(
                    vE[:, b * 16 + rt, :, 0:HD],
                    psv[:, :].rearrange("p (h c) -> p h c", c=HD))

            # ---- attention units (qb, pair, kp), software-pipelined ----
            units = [(qb, pair, kp)
                     for qb in range(4)
                     for pair in range(HL)
                     for kp in range(2 * (qb + 1))]
            NU = len(units)
            uidx = {u: i for i, u in enumerate(units)}
            st_tiles = {}   # t -> [stps_u0, stps_u1]
            av_tiles = {}   # (qb, pair) -> [avs_u0, avs_u1]

            def emit_st(t):
                qb, pair, kp = units[t]
                n_kp = 2 * (qb + 1)
                q0 = qb * 512
                o = kp - (n_kp - 2)  # diag pair offset; >=0 on diagonal
                qv0 = 256 if o == 1 else 0
                stps = [st_ps.tile([128, 2, 512], f32, tag="st",
                                   name=f"st{u}") for u in range(2)]
                st_tiles[t] = stps
                for i in range(2):
                    for u in range(2):
                        kt = 2 * kp + i
                        nc.tensor.matmul(
                            stps[u][:, i, qv0:512],
                            lhsT=kT[u * 64:(u + 1) * 64, pair,
                                    kt * 128:(kt + 1) * 128],
                            rhs=qT[u * 64:(u + 1) * 64, pair,
                                   q0 + qv0:q0 + 512],
                            start=True, stop=True)

            def emit_sm_av(t):
                qb, pair, kp = units[t]
                n_kp = 2 * (qb + 1)
                o = kp - (n_kp - 2)
                qv0 = 256 if o == 1 else 0
                if kp == 0:
                    av_tiles[(qb, pair)] = [
                        av_ps.tile([128, 512], f32, tag="av", name=f"av{u}")
                        for u in range(2)]
                avs = av_tiles[(qb, pair)]
                stps = st_tiles.pop(t)
                for u in range(2):
                    pt = ptp.tile([128, 2, 512], bf16, tag="pt")
                    nc.scalar.activation(pt[:, :, qv0:512],
                                         stps[u][:, :, qv0:512], EXP,
                                         scale=float(HD) ** -0.5)
                    if o >= 0:
                        mk2 = masks[:, o, :].rearrange("p (i c) -> p i c",
                                                       i=2)
                        nc.vector.tensor_mul(pt[:, :, qv0:512],
                                             pt[:, :, qv0:512],
                                             mk2[:, :, qv0:512])
                    for i in range(2):
                        kt = 2 * kp + i
                        nc.tensor.matmul(
                            avs[u][:, qv0:512],
                            lhsT=vE[:, u * 16 + kt, pair, :],
                            rhs=pt[:, i, qv0:512],
                            start=(kp == 0 and i == 0),
                            stop=(kp == n_kp - 1 and i == 1))

            def emit_norm(qb, pair):
                avs = av_tiles.pop((qb, pair))
                a2v = ab_in[qb].rearrange("(j f) c -> f j c", f=192)
                a2v = a2v[64 * pair:64 * (pair + 1), :, :]
                for u in range(2):
                    lsb = sm.tile([64, 512], f32, tag="lsb", name=f"lsb{u}")
                    nc.vector.tensor_copy(lsb[:], avs[u][64:128, :])
                    rec = sm.tile([64, 512], f32, tag="rec", name=f"rec{u}")
                    nc.vector.reciprocal_approx_fast(rec[:], lsb[:])
                    ctxn = sm.tile([64, 512], bf16, tag="ctxn",
                                   name=f"ctxn{u}")
                    nc.vector.tensor_mul(ctxn[:], avs[u][0:64, :], rec[:])
                    nc.sync.dma_start(
                        a2v[:, 4 * u:4 * u + 4, :],
                        ctxn.rearrange("p (q c) -> p q c", q=4))

            def emit_coll(ins, outs):
                nc.gpsimd.collective_compute(
                    "AllToAll", mybir.AluOpType.bypass,
                    ins=[ins[:]], outs=[outs[:]], replica_groups=RG)

            def fetch_ag(ag, qb):
                nc.sync.dma_start(
                    ag[:], ab_out[qb].rearrange("(o p) r -> p o r", p=128))

            def emit_outproj_blk(qb, blk, ag, split_dma=False):
                osb = outp.tile([128, D], f32, tag="osb")
                for nh in range(2):
                    po = mix_ps.tile([128, 384], f32, tag="mix", name="po")
                    n0 = nh * 384
                    for j in range(KSUB):
                        nc.tensor.matmul(po[:],
                                         lhsT=ag[:, 6 * blk + j, :],
                                         rhs=wp[:, j, n0:n0 + 384],
                                         start=(j == 0),
                                         stop=(j == KSUB - 1))
                    nc.vector.tensor_add(osb[:, n0:n0 + 384], po[:],
                                         biasB[:, n0:n0 + 384])
                    if split_dma:
                        nc.sync.dma_start(out_d[qb, blk, :, n0:n0 + 384],
                                          osb[:, n0:n0 + 384])
                if not split_dma:
                    nc.sync.dma_start(out_d[qb, blk], osb[:])

            # ---- software-pipelined emission ---------------------------
            # warmup collective: absorb ncfw first-call overhead during proj
            nc.sync.dma_start(warm_in[0:128, :], masks[:, 0, 0:8])
            emit_coll(warm_in, warm_out)
            # prologue: everything attention qb0 needs
            for ct in range(3):
                emit_qk_ct(0, 0, ct)
            for ct in range(3):
                emit_qk_ct(1, 0, ct)
            for rt in range(4):
                emit_v_tile(0, rt)
                emit_v_tile(1, rt)

            # filler queue: (deadline=emission step, emit_fn); before each
            # unit's S_T all fillers due by then are drained (hard ordering
            # requirement: a filler must be emitted before the attention
            # that consumes its output), plus one opportunistically per
            # step to spread PE filler.
            import heapq
            fqh = []
            fqseq = [0]

            def fq_push(dl, fn):
                heapq.heappush(fqh, (dl, fqseq[0], fn))
                fqseq[0] += 1

            for rc in range(1, 4):
                for b in range(2):
                    for ct in range(3):
                        # qT/kT rows rc needed from the first unit of qb=rc;
                        # its S_T is emitted one step early
                        fq_push(uidx[(rc, 0, 0)] - 1,
                                lambda b=b, rc=rc, ct=ct:
                                emit_qk_ct(b, rc, ct))
                for rt in range(4 * rc, 4 * rc + 4):
                    for b in range(2):
                        # vE row-tile rt consumed at AV of kp=rt//2 of the
                        # first pair of qb=rc
                        fq_push(uidx[(rc, 0, max(0, rt // 2 - 1))],
                                lambda b=b, rt=rt: emit_v_tile(b, rt))

            def drain(n, due=None):
                k = 0
                while fqh and (k < n or (due is not None and fqh[0][0] <= due)):
                    heapq.heappop(fqh)[2]()
                    k += 1

            # out-projection of q-block qb: ag fetch ~half a q-block after
            # AB(qb) completes (its DMA is queue-gated on AB(qb) anyway);
            # the matmul blocks run as PE filler another half-block later
            FETCH_AT = {0: (1, 2, 1), 1: (2, 1, 1), 2: (3, 0, 2)}
            OP_AT = {0: [(2, 0, 2), (2, 0, 4)],
                     1: [(3, 0, 4), (3, 0, 6)],
                     2: [(3, 1, 2), (3, 1, 5)]}
            ags = {}

            emit_st(0)
            for t in range(NU):
                if t + 1 < NU:
                    drain(1, due=t + 1)
                    emit_st(t + 1)
                emit_sm_av(t)
                qb, pair, kp = units[t]
                if kp != 2 * (qb + 1) - 1:
                    continue
                # ---- last k-pair of (qb, pair) ----
                emit_norm(qb, pair)
                if pair == 2 and qb == 3:
                    drain(99, due=NU)   # everything out before the last coll
                if pair == 2:
                    emit_coll(ab_in[qb], ab_out[qb])
                    if qb < 3:
                        ag = agp.tile([128, 2 * KSUB, 128], bf16, tag="ag",
                                      name=f"ag{qb}")
                        ags[qb] = ag
                        fq_push(uidx[FETCH_AT[qb]],
                                lambda ag=ag, qb=qb: fetch_ag(ag, qb))
                        for blk, su in enumerate(OP_AT[qb]):
                            fq_push(uidx[su],
                                    lambda qb=qb, blk=blk, ag=ag:
                                    emit_outproj_blk(qb, blk, ag))
            # ---- epilogue: q-block 3 out-projection ---------------------
            ag3 = agp.tile([128, 2 * KSUB, 128], bf16, tag="ag", name="ag3")
            fetch_ag(ag3, 3)
            emit_outproj_blk(3, 0, ag3, split_dma=True)
            emit_outproj_blk(3, 1, ag3, split_dma=True)

    nc.compile()
    return nc


def _get_nc():
    if "nc" not in _CACHE:
        _CACHE["nc"] = _build_nc()
    return _CACHE["nc"]


def _masks_np():
    k = np.arange(128)[:, None]
    q = np.arange(512)[None, :]
    tiles = [(q >= k + 128 * t) for t in range(4)]
    m = np.stack([np.concatenate([tiles[2 * o], tiles[2 * o + 1]], axis=1)
                  for o in range(2)])
    return m.astype(BF16)


def _prep_in_maps(x, Wq, Wk, Wv, Wp, bp):
    x = np.asarray(x, dtype=np.float32)
    mk = _masks_np()
    # Natural Wp row order: the merged per-q-block AllToAll delivers each
    # sender's 192 rows contiguously, and 4 senders x 192 = 6 x 128-row
    # matmul chunks in plain head order.
    wp_full = np.asarray(Wp).astype(BF16)
    bp_row = np.asarray(bp, dtype=np.float32).reshape(1, D).astype(BF16)
    xT_bg = []
    for bg in range(2):
        xl = x[2 * bg:2 * bg + 2].reshape(R, D)
        xT_bg.append(np.ascontiguousarray(xl.T).astype(BF16))
    wqk_hg, wv_hg = [], []
    for hg in range(4):
        hs = slice(192 * hg, 192 * (hg + 1))
        wqk_hg.append(np.concatenate(
            [np.asarray(Wq)[:, hs], np.asarray(Wk)[:, hs]], axis=1).astype(BF16))
        wv_hg.append(np.asarray(Wv)[:, hs].astype(BF16))
    in_maps = []
    for c in range(N_CORES):
        bg, hg = c // 4, c % 4
        in_maps.append({
            "xT": xT_bg[bg],
            "wqk": wqk_hg[hg],
            "wv": wv_hg[hg],
            "wp": wp_full,
            "bp": bp_row,
            "mk": mk,
        })
    return in_maps


def kernel(x, Wq, Wk, Wv, Wp, bp):
    from concourse import bass_utils

    nc = _get_nc()
    in_maps = _prep_in_maps(x, Wq, Wk, Wv, Wp, bp)
    res = bass_utils.run_bass_kernel_spmd(nc, in_maps,
                                          core_ids=list(range(N_CORES)))
    out = np.empty((B, S, D), np.float32)
    for c in range(N_CORES):
        sh = res.results[c]["out"]  # [4 chunks, 2 blocks, 128, D]
        for qb in range(4):
            for blk in range(2):
                batch = 2 * blk + c // 4
                s0 = 512 * qb + 128 * (c % 4)
                out[batch, s0:s0 + 128] = sh[qb, blk]
    return out
